# revision 88
# baseline (speedup 1.0000x reference)
"""Trainium2 Bass kernel for nn_Attention_82867099009253 (sparse_attention).

Tensor-parallel over heads (H=8 == 8 NeuronCores); each core computes one
head for all 4 batches:
  kv_in = depthwise_conv3(x^T) (chunked @1000, zero-pad) + x^T   [DVE engine]
  q = (Wq_h @ x^T) * hd^-0.5        (scale folded into host-side weights)
  k|v = [Wk_h; Wv_h] @ kv_in        (single-tap fused projection)
  S^T[k,m] = k^T q + rpe^T          (per 128-key chunk, psum f32; the rpe
                                     bias rides the same psum accumulation as
                                     an fp8e4m3 DoubleRow identity-matmul:
                                     tile0 = I*rpe_chunk, tile1 = 0*next)
  P^T = exp(S^T - 4)                (fp16, ACT engine; softmax max-
                                     subtraction skipped: |S|<~8)
  out[m,d] += P_chunk^T^T... PV computed TRANSPOSED: stationary = P^T
  chunk [keys, m-128], moving = v_aug [keys, 65] (v^T columns + ones col
  for the denominator) -> psum [m, 65]. v^T built by DMA-XBAR transposes.
  Host divides num/den and reassembles; the reference's flat reshape makes
  each head's [hd, L] block contiguous in the output.
All matmuls fp16 except the rpe-add (fp8, exact for the identity path and
3.6% relative on |rpe|<=0.1 values -> ~1e-3 effect on S).
"""

import collections
import os
import numpy as np
import ml_dtypes

import concourse.bass as bass
import concourse.bacc as bacc
import concourse.tile as tile
import concourse.mybir as mybir
from concourse.bass_utils import run_bass_kernel_spmd

F32 = mybir.dt.float32
F16 = mybir.dt.float16
F8 = mybir.dt.float8e4
Alu = mybir.AluOpType
Act = mybir.ActivationFunctionType
DR = mybir.MatmulPerfMode.DoubleRow
E4 = ml_dtypes.float8_e4m3

B, L, C, H = 4, 2000, 512, 8
HD = C // H            # 64
CH = 1000              # conv chunk
PW = 2 * CH + 4        # padded x width: [0 | ch0 | 0 0 | ch1 | 0]
NCH = 16               # 128-row key chunks (15*128 + 80)
NDR = 8                # chunks 0..NDR-1 add rpe via fp8 DoubleRow on PE;
                       # chunks NDR.. multiply exp(rpe) on DVE (load balance)
EXPB = -4.0            # exp bias (p = exp(S + rpe + EXPB); cancels in ratio)
# m-halves: (m offset, width, S-matmul piece widths, PVT m-chunk widths)
MH = [(0, 1024, [(0, 512), (512, 512)], [128] * 8),
      (1024, 976, [(0, 512), (512, 464)], [128] * 7 + [80])]

LAST_EXEC_NS = None
LAST_RESULTS = None


def _cw(n):
    return 128 if n < NCH - 1 else L - 128 * (NCH - 1)


def _center_col(off):
    ch = off // CH
    return 1 + ch * (CH + 2) + (off - ch * CH)


def build_kernel(debug=False, repeat=1):
    nc = bacc.Bacc("TRN2")

    xpad_d = nc.dram_tensor("xpad", [B, C, PW], F16, kind="ExternalInput")
    rpe8_d = nc.dram_tensor("rpe8", [NCH, 128, L], F8, kind="ExternalInput")
    e16_d = nc.dram_tensor("e16", [NCH - NDR, 128, L], F16, kind="ExternalInput")
    iz_d = nc.dram_tensor("iz", [128, 2, 128], F8, kind="ExternalInput")
    zi_d = nc.dram_tensor("zi", [128, 2, 128], F8, kind="ExternalInput")
    wq_d = nc.dram_tensor("wqT", [C, HD], F16, kind="ExternalInput")
    wkv_d = nc.dram_tensor("wkvT", [C, 128], F16, kind="ExternalInput")
    wkv3_d = nc.dram_tensor("wkv3T", [3, C, 128], F16, kind="ExternalInput")
    cwm_d = nc.dram_tensor("convw", [128, 12], F32, kind="ExternalInput")
    bq_d = nc.dram_tensor("biasq", [HD, 1], F32, kind="ExternalInput")
    bkv_d = nc.dram_tensor("biaskv", [128, 1], F32, kind="ExternalInput")
    out_d = nc.dram_tensor("outT", [B, 128, NCH, 65], F32, kind="ExternalOutput")
    if debug:
        kk_dbg = nc.dram_tensor("kk_dbg", [128, L], F16, kind="ExternalOutput")
        qq_dbg = nc.dram_tensor("qq_dbg", [128, L], F16, kind="ExternalOutput")
        vt_dbg = nc.dram_tensor("vt_dbg", [128, 2048], F16, kind="ExternalOutput")
        vb_dbg = nc.dram_tensor("vb_dbg", [128, NCH, 65], F16,
                                kind="ExternalOutput")
        pt_dbg = nc.dram_tensor("pt_dbg", [2, 128, 1024], F16,
                                kind="ExternalOutput")

    with tile.TileContext(nc) as tc:
        with (
            tc.tile_pool(name="const", bufs=1) as const,
            tc.tile_pool(name="xp", bufs=5) as xp_pool,
            tc.tile_pool(name="cvp", bufs=6) as cv_pool,
            tc.tile_pool(name="act2k", bufs=2) as act2k,
            tc.tile_pool(name="vb", bufs=2) as vb_pool,
            tc.tile_pool(name="pt", bufs=12) as pt_pool,
            tc.tile_pool(name="ob", bufs=2) as ob_pool,
            tc.tile_pool(name="ppp", bufs=2, space="PSUM") as pp,    # 2 banks
            tc.tile_pool(name="stp", bufs=2, space="PSUM") as stp,   # 4 banks
            tc.tile_pool(name="pvp", bufs=1, space="PSUM") as pvp,   # 2 banks
        ):
            # ---- persistent constants ----
            wq_sb = const.tile([128, 4, HD], F16)
            nc.sync.dma_start(wq_sb[:], wq_d[:].rearrange("(o p) d -> p o d", p=128))
            wkv_sb = const.tile([128, 4, 128], F16)
            # 3-tap folded weights, used only for batch 0's projection (no
            # DVE-conv dependency in the prologue); split DMAs so the first
            # projection matmuls aren't gated on one 17us transfer
            wkv3_sb = const.tile([128, 3, 4, 128], F16)
            nc.sync.dma_start(
                wkv3_sb[:], wkv3_d[:].rearrange("t (o p) d -> p t o d", p=128))
            zrow = const.tile([1, 512], F16)
            nc.vector.memset(zrow[:], 0.0)
            zcol = const.tile([1, 128], F16)
            nc.vector.memset(zcol[:], 0.0)
            bq_sb = const.tile([HD, 1], F32)
            nc.sync.dma_start(bq_sb[:], bq_d[:])
            bkv_sb = const.tile([128, 1], F32)
            nc.sync.dma_start(bkv_sb[:], bkv_d[:])
            nbias = const.tile([128, 1], F32)
            nc.vector.memset(nbias[:], EXPB)
            cw_sb = const.tile([128, 12], F32)
            iz_sb = const.tile([128, 2, 128], F8)
            zi_sb = const.tile([128, 2, 128], F8)
            rpe8 = const.tile([128, NCH, L], F8)
            e16_sb = const.tile([128, NCH - NDR, L], F16)
            wkv_sb2 = wkv_sb

            def load_late():
                # issued after batch 0's xpad so the prologue projection
                # isn't starved on the serialized DMA timeline; ordered by
                # first-use time (rpe8 paces batch 0's DoubleRow chunks)
                nc.sync.dma_start(iz_sb[:], iz_d[:])
                nc.sync.dma_start(zi_sb[:], zi_d[:])
                nc.sync.dma_start(cw_sb[:], cwm_d[:])
                nc.sync.dma_start(
                    wkv_sb2[:], wkv_d[:].rearrange("(o p) d -> p o d", p=128))
                for n in range(NCH):
                    nc.sync.dma_start(rpe8[:, n, :], rpe8_d[n])

            def load_late2():
                # after batch 1's xpad: e16 m-half-0 columns (needed by batch
                # 0's E-mult chunks ~t+20), then the half-1 columns
                for n in range(NCH - NDR):
                    nc.sync.dma_start(e16_sb[:, n, 0:1024], e16_d[n, :, 0:1024])
                for n in range(NCH - NDR):
                    nc.sync.dma_start(e16_sb[:, n, 1024:L],
                                      e16_d[n, :, 1024:L])

            def emit_conv_proj(b, pe_conv=False, _ctr=[0]):
                """Conv (DVE) + projections (PE). DMAs issued immediately;
                returns (tiles, steps) to interleave into attention chunks.
                pe_conv=True folds the conv into 3-tap PE matmuls (used for
                the first batch, where there is no other PE work to hide the
                DVE conv latency behind)."""
                _ctr[0] += 1
                u = _ctr[0]
                xts = []
                for c in range(4):
                    xt = xp_pool.tile([128, PW], F16, tag="xp", name=f"xt{u}_{c}")
                    if not pe_conv:
                        nc.sync.dma_start(xt[:],
                                          xpad_d[b, 128 * c:128 * c + 128, :])
                    xts.append(xt)
                if pe_conv:
                    # batch 0: span-0 columns of all c-tiles first, so the
                    # first kv/q l-chunks unblock ~4us earlier
                    for piece in range(2):
                        for c in range(4):
                            nc.sync.dma_start(
                                xts[c][:, 1002 * piece:1002 * piece + 1002],
                                xpad_d[b, 128 * c:128 * c + 128,
                                       1002 * piece:1002 * piece + 1002])

                kk = act2k.tile([HD, L], F16, tag="kk", name=f"kk{u}")
                vt = act2k.tile([128, 2048], F16, tag="vt", name=f"vt{u}")
                qq = act2k.tile([HD, L], F16, tag="qq", name=f"qq{u}")
                v_big = vb_pool.tile([128, NCH, 65], F16, tag="vb", name=f"vb{u}")
                kv_in = []
                steps = []

                def conv_chunk(c):
                    # kv_in[c][:, sp, l] = conv3(x)[l of span sp] (+x
                    # residual, folded into w2'); spans are the two
                    # zero-padded halves. Returns per-span step triples so
                    # the DVE work can spread across attention chunk slots.
                    t = cv_pool.tile([128, 2, CH], F16, tag="cv",
                                     name=f"cv{u}_{c}")
                    s = cv_pool.tile([128, 2, CH], F16, tag="cvs",
                                     name=f"cvs{u}_{c}")
                    kv_in.append(t)
                    # span sp of xpad occupies cols [1002sp, 1002sp+1002);
                    # conv position l of span sp, tap d reads col 1002sp+l+d
                    xtv = xts[c][:].rearrange("p (s w) -> p s w", w=CH + 2)
                    xv = [xtv[:, :, d:d + CH] for d in range(3)]
                    w1 = cw_sb[:, 3 * c:3 * c + 1]
                    w2 = cw_sb[:, 3 * c + 1:3 * c + 2]
                    w3 = cw_sb[:, 3 * c + 2:3 * c + 3]

                    def span_steps(sp):
                        tv, sv = t[:, sp, :], s[:, sp, :]
                        xl = [v[:, sp, :] for v in xv]
                        return [
                            lambda: (
                                nc.vector.tensor_scalar(tv, xl[1], w2, None,
                                                        Alu.mult),
                                nc.vector.tensor_scalar(sv, xl[0], w1, None,
                                                        Alu.mult)),
                            lambda: (
                                nc.vector.tensor_tensor(out=tv, in0=tv,
                                                        in1=sv, op=Alu.add),
                                nc.vector.tensor_scalar(sv, xl[2], w3, None,
                                                        Alu.mult)),
                            lambda: nc.vector.tensor_tensor(
                                out=tv, in0=tv, in1=sv, op=Alu.add),
                        ]
                    return span_steps

                def kv_lchunk(li):
                    # one 500-col l-chunk per 1-bank psum tile (pp bufs=2
                    # double-buffers the mms of chunk li+1 with the copy of
                    # chunk li)
                    ps_kv = pp.tile([128, 512], F32, tag="pp",
                                    name=f"pskv{u}_{li}")
                    lo = 500 * li

                    def mms(ps_kv=ps_kv, lo=lo):
                        if pe_conv:
                            cc = _center_col(lo)
                            for t in range(3):
                                for c in range(4):
                                    nc.tensor.matmul(
                                        ps_kv[:, 0:500],
                                        wkv3_sb[:, t, c, :],
                                        xts[c][:, cc - 1 + t:cc - 1 + t + 500],
                                        start=(t == 0 and c == 0),
                                        stop=(t == 2 and c == 3),
                                    )
                            return
                        h, loc = divmod(lo, 1000)
                        for c in range(4):
                            nc.tensor.matmul(
                                ps_kv[:, 0:500],
                                wkv_sb[:, c, :],
                                kv_in[c][:, h, loc:loc + 500],
                                start=(c == 0), stop=(c == 3),
                            )

                    def copy(ps_kv=ps_kv, lo=lo):
                        nc.vector.tensor_scalar(
                            kk[:, lo:lo + 500], ps_kv[0:HD, 0:500],
                            bkv_sb[0:HD], None, Alu.add)
                        nc.vector.tensor_scalar(
                            vt[HD:128, lo:lo + 500], ps_kv[HD:128, 0:500],
                            bkv_sb[HD:128], None, Alu.add)
                    return [mms, copy]

                def q_lchunk(li):
                    # same psum tag as kv so both share one 2-deep bank ring
                    ps_q = pp.tile([128, 512], F32, tag="pp",
                                   name=f"psq{u}_{li}")
                    lo = 500 * li

                    def mms(ps_q=ps_q, lo=lo):
                        cc = _center_col(lo)
                        for c in range(4):
                            nc.tensor.matmul(
                                ps_q[0:HD, 0:500],
                                wq_sb[:, c, :],
                                xts[c][:, cc:cc + 500],
                                start=(c == 0), stop=(c == 3),
                            )

                    def copy(ps_q=ps_q, lo=lo):
                        nc.vector.tensor_scalar(
                            qq[:, lo:lo + 500], ps_q[0:HD, 0:500], bq_sb[:],
                            None, Alu.add)
                    return [mms, copy]

                def vtr(v_big=v_big):
                    # one XBAR transpose builds v^T for all chunks at once:
                    # the [128,16,64] dest maps as vb64[p, n, d] =
                    # v[d, 128n + p]. Lands in a packed 4B-aligned tile, then
                    # a strided copy places it next to the ones column.
                    vb64 = cv_pool.tile([128, NCH, 64], F16, tag="vb64",
                                        name=f"vb64{u}")
                    nc.gpsimd.memset(v_big[:, :, 64:65], 1.0)
                    nc.gpsimd.memset(vt[HD:128, 2000:2048], 0.0)
                    nc.sync.dma_start_transpose(vb64[:], vt[HD:128, :])
                    nc.vector.tensor_scalar(
                        v_big[:, :, 0:64], vb64[:], 0.0, None, Alu.add)

                if pe_conv:
                    for li in range(4):
                        steps += kv_lchunk(li)
                    for li in range(4):
                        mq, cq = q_lchunk(li)
                        steps.append(lambda mq=mq, cq=cq: (mq(), cq()))
                    steps.append(vtr)
                else:
                    # conv span-0 first (DVE-only: waiting on this batch's
                    # xpad can't head-of-line block the in-order PE), then q
                    # once xpad has surely landed, kv l-chunks when their
                    # conv spans are done, v transpose last (first-half PVTs
                    # are lagged behind it)
                    convs = [conv_chunk(c) for c in range(4)]
                    for c in range(4):
                        steps += convs[c](0)
                    for li in range(4):
                        mq, cq = q_lchunk(li)
                        steps.append(lambda mq=mq, cq=cq: (mq(), cq()))
                    steps += kv_lchunk(0)
                    steps += kv_lchunk(1)
                    for c in range(4):
                        steps += convs[c](1)
                    steps += kv_lchunk(2)
                    steps += kv_lchunk(3)
                    steps.append(vtr)
                return (kk, qq, v_big), steps

            # PVT emission is lagged behind S/exp emission via this deque so
            # the in-order PE never head-of-line blocks on an exp that hasn't
            # drained (each entry = one chunk's PVT matmul group)
            pending = collections.deque()

            def emit_attention_half(b, mh_i, kk, qq, v_big, ot, fillers=None,
                                    lag=1, rev=False, ndr=NDR, _ctr=[0]):
                mo0, mw0, mms, mcw = MH[mh_i]
                fillers = fillers if fillers is not None else []
                _ctr[0] += 1
                u = _ctr[0]
                ps_out = pvp.tile([128, 2, 512], F32, tag="pv", name=f"po{u}")
                zeroed = [False]

                def ensure_zero():
                    # zero-init each psum bank with a zero-stationary matmul:
                    # runs on PE (in-order with the PVT accumulation) and its
                    # full-bank write gives every start=False PVT matmul a
                    # tracked RAW dependency
                    if not zeroed[0]:
                        zeroed[0] = True
                        for q4 in range(2):
                            nc.tensor.matmul(ps_out[:, q4, :], zcol[:],
                                             zrow[:], start=True, stop=True,
                                             skip_group_check=True)

                # rev=True ends on a DoubleRow chunk (no DVE multiply in the
                # tail dependency chain) — used for the final half's drain
                order = list(reversed(range(NCH))) if rev else list(range(NCH))
                for idx, n in enumerate(order):
                    w = _cw(n)
                    st = stp.tile([128, 1024], F32, tag="st", name=f"st{u}_{n}")
                    for mo, mw in mms:
                        nc.tensor.matmul(
                            st[0:w, mo:mo + mw],
                            kk[:, 128 * n:128 * n + w],
                            qq[:, mo0 + mo:mo0 + mo + mw],
                            start=True, stop=(n >= ndr),
                        )
                    if n < ndr:
                        # rpe bias add: fp8 DoubleRow identity matmul, k-tile
                        # pair (n, n+1) with second tile zero-weighted (IZ),
                        # or (n-1, n) via ZI for the last DR chunk
                        lhs8 = iz_sb if n < ndr - 1 else zi_sb
                        n0 = n if n < ndr - 1 else n - 1
                        for mo, mw in mms:
                            nc.tensor.matmul(
                                st[0:w, mo:mo + mw],
                                lhs8[:, :, 0:w],
                                rpe8[:, n0:n0 + 2, mo0 + mo:mo0 + mo + mw],
                                start=False, stop=True, perf_mode=DR,
                                skip_group_check=True,
                            )
                    # drain gradually toward `lag` (at most 2 per chunk) so a
                    # lag transition doesn't batch PE work ahead of the next
                    # S matmul
                    target = max(lag, len(pending) - 2)
                    while len(pending) > target:
                        pending.popleft()()
                    pt = pt_pool.tile([128, 1024], F16, tag="pt", name=f"pt{u}_{n}")
                    nc.scalar.activation(
                        pt[0:w, 0:mw0], st[0:w, 0:mw0], Act.Exp,
                        bias=nbias[0:w])
                    if n >= ndr:
                        # rpe applied multiplicatively: pt *= exp(rpe) (DVE)
                        nc.vector.tensor_tensor(
                            out=pt[0:w, 0:mw0], in0=pt[0:w, 0:mw0],
                            in1=e16_sb[0:w, n - NDR, mo0:mo0 + mw0],
                            op=Alu.mult)
                    if debug and b == 0 and mh_i == 0 and n < 2:
                        nc.sync.dma_start(pt_dbg[n], pt[:])

                    def pvt(n=n, w=w, pt=pt, ps_out=ps_out,
                            last=(idx == NCH - 1)):
                        ensure_zero()
                        # transposed PV: stationary = pt m-chunk, moving = v_aug
                        for j, mp in enumerate(mcw):
                            q4, j4 = divmod(j, 4)
                            nc.tensor.matmul(
                                ps_out[0:mp, q4, 65 * j4:65 * j4 + 65],
                                pt[0:w, 128 * j:128 * j + mp],
                                v_big[0:w, n, :],
                                start=False, stop=last,
                                skip_group_check=True,
                            )
                        if last:
                            for q4 in range(2):
                                nc.vector.tensor_scalar(
                                    ot[:, 8 * mh_i + 4 * q4:
                                       8 * mh_i + 4 * q4 + 4, :],
                                    ps_out[:, q4, 0:260].rearrange(
                                        "p (c w) -> p c w", w=65),
                                    0.0, None, Alu.add)
                            nc.sync.dma_start(
                                out_d[b, :, 8 * mh_i:8 * mh_i + 8, :],
                                ot[:, 8 * mh_i:8 * mh_i + 8, :])
                    pending.append(pvt)
                    # interleave one next-batch conv/projection step per chunk
                    if fillers:
                        fillers.pop(0)()

            state, steps0 = emit_conv_proj(0, pe_conv=True)
            # prologue: run just enough projection for attention to start
            # (kv l-chunk 0 -> keys 0..499; q l-chunks 0..2 -> queries
            # 0..1500 covering m-half 0); the rest become fillers, with the
            # v^T transpose last and the first half's PVTs lagged behind it
            pre = steps0[0:2] + steps0[8:11]
            fill0 = steps0[2:8] + steps0[11:]
            for st_fn in pre:
                st_fn()
            load_late()
            for rep in range(repeat):
                for b in range(B):
                    kk, qq, v_big = state
                    ot = ob_pool.tile([128, NCH, 65], F32, tag="ot",
                                      name=f"ot{b}_{rep}")
                    if b + 1 < B or rep + 1 < repeat:
                        state, fillers = emit_conv_proj((b + 1) % B)
                    else:
                        fillers = []
                    first = (rep == 0 and b == 0)
                    if first:
                        load_late2()
                    # batch 0's first half is already resource-crunched:
                    # only its own remaining projection steps pop there;
                    # batch 1's steps wait for the second half
                    emit_attention_half(b, 0, kk, qq, v_big, ot,
                                        fill0 if first else fillers,
                                        lag=8, ndr=NCH if first else NDR)
                    final = b + 1 >= B and rep + 1 >= repeat
                    emit_attention_half(b, 1, kk, qq, v_big, ot, fillers,
                                        lag=1, rev=final,
                                        ndr=NCH if first else NDR)
                    for st_fn in fillers:
                        st_fn()
            while pending:
                pending.popleft()()
            if debug:
                nc.sync.dma_start(kk_dbg[:], state[0][:])
                nc.sync.dma_start(qq_dbg[:], state[1][:])
                nc.sync.dma_start(vb_dbg[:], state[2][:])

    nc.finalize()
    return nc


_NC_CACHE = None


def _get_nc():
    global _NC_CACHE
    if _NC_CACHE is None:
        _NC_CACHE = build_kernel()
    return _NC_CACHE


def _host_prep(x, rpe, Wq, bq, Wkv, bkv, Wl, bl):
    scale = float(HD) ** -0.5
    xt = np.ascontiguousarray(np.swapaxes(x, 1, 2))          # [B, C, L]
    xpad = np.zeros((B, C, PW), np.float16)
    xpad[:, :, 1:1 + CH] = xt[:, :, 0:CH]
    xpad[:, :, CH + 3:CH + 3 + CH] = xt[:, :, CH:L]

    w1 = Wl[:, 0, 0].astype(np.float64)
    w2 = Wl[:, 0, 1].astype(np.float64) + 1.0
    w3 = Wl[:, 0, 2].astype(np.float64)
    convw = np.zeros((128, 12), np.float32)
    for c in range(4):
        sl = slice(128 * c, 128 * c + 128)
        convw[:, 3 * c + 0] = w1[sl]
        convw[:, 3 * c + 1] = w2[sl]
        convw[:, 3 * c + 2] = w3[sl]

    bias_kv_full = (Wkv.astype(np.float64) @ bl.astype(np.float64)
                    + bkv.astype(np.float64))

    iz = np.zeros((128, 2, 128), E4)
    zi = np.zeros((128, 2, 128), E4)
    iz[:, 0][np.arange(128), np.arange(128)] = 1.0
    zi[:, 1][np.arange(128), np.arange(128)] = 1.0

    in_maps = []
    for h in range(H):
        r = slice(HD * h, HD * h + HD)
        rv = slice(C + HD * h, C + HD * h + HD)
        wqT = np.ascontiguousarray((Wq[r, :] * scale).T).astype(np.float16)
        wsel = np.concatenate([Wkv[r, :], Wkv[rv, :]], 0).astype(np.float64)
        wkvT = np.ascontiguousarray(wsel.T).astype(np.float16)
        taps = [w1, w2, w3]
        wkv3T = np.stack(
            [np.ascontiguousarray((wsel * taps[t][None, :]).T) for t in range(3)],
            0).astype(np.float16)
        biasq = (bq[r] * scale).astype(np.float32).reshape(HD, 1)
        biaskv = np.concatenate(
            [bias_kv_full[r], bias_kv_full[rv]]).astype(np.float32).reshape(128, 1)
        rpeT = np.zeros((NCH * 128, L), np.float32)
        rpeT[0:L] = rpe[0, h].T
        rpeTc = rpeT.reshape(NCH, 128, L)
        rpe8 = rpeTc.astype(E4)
        e16 = np.exp(rpeTc[NDR:NCH]).astype(np.float16)
        in_maps.append({
            "xpad": xpad, "rpe8": rpe8, "e16": e16, "iz": iz, "zi": zi,
            "wqT": wqT, "wkvT": wkvT, "wkv3T": wkv3T, "convw": convw,
            "biasq": biasq, "biaskv": biaskv,
        })
    return in_maps


def kernel(x, relative_pos_enc, Wq, bq, Wkv, bkv, Wl, bl):
    global LAST_EXEC_NS, LAST_RESULTS
    in_maps = _host_prep(np.asarray(x, np.float32),
                         np.asarray(relative_pos_enc, np.float32),
                         np.asarray(Wq, np.float32), np.asarray(bq, np.float32),
                         np.asarray(Wkv, np.float32), np.asarray(bkv, np.float32),
                         np.asarray(Wl, np.float32), np.asarray(bl, np.float32))
    nc = _get_nc()
    trace = bool(int(os.environ.get("KERNEL_TRACE", "0")))
    res = run_bass_kernel_spmd(nc, in_maps, core_ids=list(range(H)), trace=trace)
    LAST_EXEC_NS = res.exec_time_ns
    LAST_RESULTS = res
    arr = np.stack([res.results[h]["outT"] for h in range(H)], 0)
    # [H, B, 128, NCH, 65]: m = 128*chunk + p -> [H, B, L, 65]
    arr = arr.transpose(0, 1, 3, 2, 4).reshape(H, B, NCH * 128, 65)[:, :, 0:L]
    out_md = arr[:, :, :, 0:64] / arr[:, :, :, 64:65]
    out_t = np.ascontiguousarray(out_md.transpose(0, 1, 3, 2))  # [H, B, 64, L]
    out = np.ascontiguousarray(out_t.transpose(1, 0, 2, 3)).reshape(B, L, C)
    return out.astype(np.float32)


# revision 89
# speedup vs baseline: 1.0187x; 1.0187x over previous
"""Trainium2 Bass kernel for nn_Attention_82867099009253 (sparse_attention).

Tensor-parallel over heads (H=8 == 8 NeuronCores); each core computes one
head for all 4 batches:
  kv_in = depthwise_conv3(x^T) (chunked @1000, zero-pad) + x^T   [DVE engine]
  q = (Wq_h @ x^T) * hd^-0.5        (scale folded into host-side weights)
  k|v = [Wk_h; Wv_h] @ kv_in        (single-tap fused projection)
  S^T[k,m] = k^T q + rpe^T          (per 128-key chunk, psum f32; the rpe
                                     bias rides the same psum accumulation as
                                     an fp8e4m3 DoubleRow identity-matmul:
                                     tile0 = I*rpe_chunk, tile1 = 0*next)
  P^T = exp(S^T - 4)                (fp16, ACT engine; softmax max-
                                     subtraction skipped: |S|<~8)
  out[m,d] += P_chunk^T^T... PV computed TRANSPOSED: stationary = P^T
  chunk [keys, m-128], moving = v_aug [keys, 65] (v^T columns + ones col
  for the denominator) -> psum [m, 65]. v^T built by DMA-XBAR transposes.
  Host divides num/den and reassembles; the reference's flat reshape makes
  each head's [hd, L] block contiguous in the output.
All matmuls fp16 except the rpe-add (fp8, exact for the identity path and
3.6% relative on |rpe|<=0.1 values -> ~1e-3 effect on S).
"""

import collections
import os
import numpy as np
import ml_dtypes

import concourse.bass as bass
import concourse.bacc as bacc
import concourse.tile as tile
import concourse.mybir as mybir
from concourse.bass_utils import run_bass_kernel_spmd

F32 = mybir.dt.float32
F16 = mybir.dt.float16
F8 = mybir.dt.float8e4
Alu = mybir.AluOpType
Act = mybir.ActivationFunctionType
DR = mybir.MatmulPerfMode.DoubleRow
E4 = ml_dtypes.float8_e4m3

B, L, C, H = 4, 2000, 512, 8
HD = C // H            # 64
CH = 1000              # conv chunk
PW = 2 * CH + 4        # padded x width: [0 | ch0 | 0 0 | ch1 | 0]
NCH = 16               # 128-row key chunks (15*128 + 80)
NDR = 8                # chunks 0..NDR-1 add rpe via fp8 DoubleRow on PE;
                       # chunks NDR.. multiply exp(rpe) on DVE (load balance)
EXPB = -4.0            # exp bias (p = exp(S + rpe + EXPB); cancels in ratio)
# m-halves: (m offset, width, S-matmul piece widths, PVT m-chunk widths)
MH = [(0, 1024, [(0, 512), (512, 512)], [128] * 8),
      (1024, 976, [(0, 512), (512, 464)], [128] * 7 + [80])]

LAST_EXEC_NS = None
LAST_RESULTS = None


def _cw(n):
    return 128 if n < NCH - 1 else L - 128 * (NCH - 1)


def _center_col(off):
    ch = off // CH
    return 1 + ch * (CH + 2) + (off - ch * CH)


def build_kernel(debug=False, repeat=1):
    nc = bacc.Bacc("TRN2")

    xpad_d = nc.dram_tensor("xpad", [B, C, PW], F16, kind="ExternalInput")
    rpe8_d = nc.dram_tensor("rpe8", [NCH, 128, L], F8, kind="ExternalInput")
    e16_d = nc.dram_tensor("e16", [NCH - NDR, 128, L], F16, kind="ExternalInput")
    iz_d = nc.dram_tensor("iz", [128, 2, 128], F8, kind="ExternalInput")
    zi_d = nc.dram_tensor("zi", [128, 2, 128], F8, kind="ExternalInput")
    wq_d = nc.dram_tensor("wqT", [C, HD], F16, kind="ExternalInput")
    wkv_d = nc.dram_tensor("wkvT", [C, 128], F16, kind="ExternalInput")
    wkv3_d = nc.dram_tensor("wkv3T", [3, C, 128], F16, kind="ExternalInput")
    cwm_d = nc.dram_tensor("convw", [128, 12], F32, kind="ExternalInput")
    bq_d = nc.dram_tensor("biasq", [HD, 1], F32, kind="ExternalInput")
    bkv_d = nc.dram_tensor("biaskv", [128, 1], F32, kind="ExternalInput")
    out_d = nc.dram_tensor("outT", [B, 128, NCH, 65], F32, kind="ExternalOutput")
    if debug:
        kk_dbg = nc.dram_tensor("kk_dbg", [128, L], F16, kind="ExternalOutput")
        qq_dbg = nc.dram_tensor("qq_dbg", [128, L], F16, kind="ExternalOutput")
        vt_dbg = nc.dram_tensor("vt_dbg", [128, 2048], F16, kind="ExternalOutput")
        vb_dbg = nc.dram_tensor("vb_dbg", [128, NCH, 65], F16,
                                kind="ExternalOutput")
        pt_dbg = nc.dram_tensor("pt_dbg", [2, 128, 1024], F16,
                                kind="ExternalOutput")

    with tile.TileContext(nc) as tc:
        with (
            tc.tile_pool(name="const", bufs=1) as const,
            tc.tile_pool(name="xp", bufs=5) as xp_pool,
            tc.tile_pool(name="cvp", bufs=6) as cv_pool,
            tc.tile_pool(name="act2k", bufs=2) as act2k,
            tc.tile_pool(name="vb", bufs=2) as vb_pool,
            tc.tile_pool(name="pt", bufs=12) as pt_pool,
            tc.tile_pool(name="ob", bufs=2) as ob_pool,
            tc.tile_pool(name="ppp", bufs=2, space="PSUM") as pp,    # 2 banks
            tc.tile_pool(name="stp", bufs=2, space="PSUM") as stp,   # 4 banks
            tc.tile_pool(name="pvp", bufs=1, space="PSUM") as pvp,   # 2 banks
        ):
            # ---- persistent constants ----
            wq_sb = const.tile([128, 4, HD], F16)
            nc.sync.dma_start(wq_sb[:], wq_d[:].rearrange("(o p) d -> p o d", p=128))
            wkv_sb = const.tile([128, 4, 128], F16)
            # 3-tap folded weights, used only for batch 0's projection (no
            # DVE-conv dependency in the prologue); split DMAs so the first
            # projection matmuls aren't gated on one 17us transfer
            wkv3_sb = const.tile([128, 3, 4, 128], F16)
            nc.sync.dma_start(
                wkv3_sb[:], wkv3_d[:].rearrange("t (o p) d -> p t o d", p=128))
            zrow = const.tile([1, 512], F16)
            nc.vector.memset(zrow[:], 0.0)
            zcol = const.tile([1, 128], F16)
            nc.vector.memset(zcol[:], 0.0)
            bq_sb = const.tile([HD, 1], F32)
            nc.sync.dma_start(bq_sb[:], bq_d[:])
            bkv_sb = const.tile([128, 1], F32)
            nc.sync.dma_start(bkv_sb[:], bkv_d[:])
            nbias = const.tile([128, 1], F32)
            nc.vector.memset(nbias[:], EXPB)
            cw_sb = const.tile([128, 12], F32)
            iz_sb = const.tile([128, 2, 128], F8)
            zi_sb = const.tile([128, 2, 128], F8)
            rpe8 = const.tile([128, NCH, L], F8)
            e16_sb = const.tile([128, NCH - NDR, L], F16)
            wkv_sb2 = wkv_sb

            def load_late():
                # issued after batch 0's xpad so the prologue projection
                # isn't starved on the serialized DMA timeline; ordered by
                # first-use time (rpe8 paces batch 0's DoubleRow chunks)
                nc.sync.dma_start(iz_sb[:], iz_d[:])
                nc.sync.dma_start(zi_sb[:], zi_d[:])
                nc.sync.dma_start(cw_sb[:], cwm_d[:])
                nc.sync.dma_start(
                    wkv_sb2[:], wkv_d[:].rearrange("(o p) d -> p o d", p=128))
                for n in range(12):
                    nc.sync.dma_start(rpe8[:, n, :], rpe8_d[n])
                for n in range(4, NCH - NDR):
                    nc.sync.dma_start(e16_sb[:, n, 0:1024],
                                      e16_d[n, :, 0:1024])

            def load_late2():
                # after batch 1's xpad: e16 m-half-0 columns (needed by batch
                # 0's E-mult chunks ~t+20), then the half-1 columns
                for n in range(4, NCH - NDR):
                    nc.sync.dma_start(e16_sb[:, n, 1024:L],
                                      e16_d[n, :, 1024:L])
                for n in range(0, 4):
                    nc.sync.dma_start(e16_sb[:, n, 0:1024], e16_d[n, :, 0:1024])
                for n in range(0, 4):
                    nc.sync.dma_start(e16_sb[:, n, 1024:L],
                                      e16_d[n, :, 1024:L])

            def emit_conv_proj(b, pe_conv=False, _ctr=[0]):
                """Conv (DVE) + projections (PE). DMAs issued immediately;
                returns (tiles, steps) to interleave into attention chunks.
                pe_conv=True folds the conv into 3-tap PE matmuls (used for
                the first batch, where there is no other PE work to hide the
                DVE conv latency behind)."""
                _ctr[0] += 1
                u = _ctr[0]
                xts = []
                for c in range(4):
                    xt = xp_pool.tile([128, PW], F16, tag="xp", name=f"xt{u}_{c}")
                    if not pe_conv:
                        nc.sync.dma_start(xt[:],
                                          xpad_d[b, 128 * c:128 * c + 128, :])
                    xts.append(xt)
                if pe_conv:
                    # batch 0: span-0 columns of all c-tiles first, so the
                    # first kv/q l-chunks unblock ~4us earlier
                    for piece in range(2):
                        for c in range(4):
                            nc.sync.dma_start(
                                xts[c][:, 1002 * piece:1002 * piece + 1002],
                                xpad_d[b, 128 * c:128 * c + 128,
                                       1002 * piece:1002 * piece + 1002])

                kk = act2k.tile([HD, L], F16, tag="kk", name=f"kk{u}")
                vt = act2k.tile([128, 2048], F16, tag="vt", name=f"vt{u}")
                qq = act2k.tile([HD, L], F16, tag="qq", name=f"qq{u}")
                v_big = vb_pool.tile([128, NCH, 65], F16, tag="vb", name=f"vb{u}")
                kv_in = []
                steps = []

                def conv_chunk(c):
                    # kv_in[c][:, sp, l] = conv3(x)[l of span sp] (+x
                    # residual, folded into w2'); spans are the two
                    # zero-padded halves. Returns per-span step triples so
                    # the DVE work can spread across attention chunk slots.
                    t = cv_pool.tile([128, 2, CH], F16, tag="cv",
                                     name=f"cv{u}_{c}")
                    s = cv_pool.tile([128, 2, CH], F16, tag="cvs",
                                     name=f"cvs{u}_{c}")
                    kv_in.append(t)
                    # span sp of xpad occupies cols [1002sp, 1002sp+1002);
                    # conv position l of span sp, tap d reads col 1002sp+l+d
                    xtv = xts[c][:].rearrange("p (s w) -> p s w", w=CH + 2)
                    xv = [xtv[:, :, d:d + CH] for d in range(3)]
                    w1 = cw_sb[:, 3 * c:3 * c + 1]
                    w2 = cw_sb[:, 3 * c + 1:3 * c + 2]
                    w3 = cw_sb[:, 3 * c + 2:3 * c + 3]

                    def span_steps(sp):
                        tv, sv = t[:, sp, :], s[:, sp, :]
                        xl = [v[:, sp, :] for v in xv]
                        return [
                            lambda: (
                                nc.vector.tensor_scalar(tv, xl[1], w2, None,
                                                        Alu.mult),
                                nc.vector.tensor_scalar(sv, xl[0], w1, None,
                                                        Alu.mult)),
                            lambda: (
                                nc.vector.tensor_tensor(out=tv, in0=tv,
                                                        in1=sv, op=Alu.add),
                                nc.vector.tensor_scalar(sv, xl[2], w3, None,
                                                        Alu.mult)),
                            lambda: nc.vector.tensor_tensor(
                                out=tv, in0=tv, in1=sv, op=Alu.add),
                        ]
                    return span_steps

                def kv_lchunk(li):
                    # one 500-col l-chunk per 1-bank psum tile (pp bufs=2
                    # double-buffers the mms of chunk li+1 with the copy of
                    # chunk li)
                    ps_kv = pp.tile([128, 512], F32, tag="pp",
                                    name=f"pskv{u}_{li}")
                    lo = 500 * li

                    def mms(ps_kv=ps_kv, lo=lo):
                        if pe_conv:
                            cc = _center_col(lo)
                            for t in range(3):
                                for c in range(4):
                                    nc.tensor.matmul(
                                        ps_kv[:, 0:500],
                                        wkv3_sb[:, t, c, :],
                                        xts[c][:, cc - 1 + t:cc - 1 + t + 500],
                                        start=(t == 0 and c == 0),
                                        stop=(t == 2 and c == 3),
                                    )
                            return
                        h, loc = divmod(lo, 1000)
                        for c in range(4):
                            nc.tensor.matmul(
                                ps_kv[:, 0:500],
                                wkv_sb[:, c, :],
                                kv_in[c][:, h, loc:loc + 500],
                                start=(c == 0), stop=(c == 3),
                            )

                    def copy(ps_kv=ps_kv, lo=lo):
                        nc.vector.tensor_scalar(
                            kk[:, lo:lo + 500], ps_kv[0:HD, 0:500],
                            bkv_sb[0:HD], None, Alu.add)
                        nc.vector.tensor_scalar(
                            vt[HD:128, lo:lo + 500], ps_kv[HD:128, 0:500],
                            bkv_sb[HD:128], None, Alu.add)
                    return [mms, copy]

                def q_lchunk(li):
                    # same psum tag as kv so both share one 2-deep bank ring
                    ps_q = pp.tile([128, 512], F32, tag="pp",
                                   name=f"psq{u}_{li}")
                    lo = 500 * li

                    def mms(ps_q=ps_q, lo=lo):
                        cc = _center_col(lo)
                        for c in range(4):
                            nc.tensor.matmul(
                                ps_q[0:HD, 0:500],
                                wq_sb[:, c, :],
                                xts[c][:, cc:cc + 500],
                                start=(c == 0), stop=(c == 3),
                            )

                    def copy(ps_q=ps_q, lo=lo):
                        nc.vector.tensor_scalar(
                            qq[:, lo:lo + 500], ps_q[0:HD, 0:500], bq_sb[:],
                            None, Alu.add)
                    return [mms, copy]

                def vtr(v_big=v_big):
                    # one XBAR transpose builds v^T for all chunks at once:
                    # the [128,16,64] dest maps as vb64[p, n, d] =
                    # v[d, 128n + p]. Lands in a packed 4B-aligned tile, then
                    # a strided copy places it next to the ones column.
                    vb64 = cv_pool.tile([128, NCH, 64], F16, tag="vb64",
                                        name=f"vb64{u}")
                    nc.gpsimd.memset(v_big[:, :, 64:65], 1.0)
                    nc.gpsimd.memset(vt[HD:128, 2000:2048], 0.0)
                    nc.sync.dma_start_transpose(vb64[:], vt[HD:128, :])
                    nc.vector.tensor_scalar(
                        v_big[:, :, 0:64], vb64[:], 0.0, None, Alu.add)

                if pe_conv:
                    for li in range(4):
                        steps += kv_lchunk(li)
                    for li in range(4):
                        mq, cq = q_lchunk(li)
                        steps.append(lambda mq=mq, cq=cq: (mq(), cq()))
                    steps.append(vtr)
                else:
                    # conv span-0 first (DVE-only: waiting on this batch's
                    # xpad can't head-of-line block the in-order PE), then q
                    # once xpad has surely landed, kv l-chunks when their
                    # conv spans are done, v transpose last (first-half PVTs
                    # are lagged behind it)
                    convs = [conv_chunk(c) for c in range(4)]
                    for c in range(4):
                        steps += convs[c](0)
                    for li in range(4):
                        mq, cq = q_lchunk(li)
                        steps.append(lambda mq=mq, cq=cq: (mq(), cq()))
                    steps += kv_lchunk(0)
                    steps += kv_lchunk(1)
                    for c in range(4):
                        steps += convs[c](1)
                    steps += kv_lchunk(2)
                    steps += kv_lchunk(3)
                    steps.append(vtr)
                return (kk, qq, v_big), steps

            # PVT emission is lagged behind S/exp emission via this deque so
            # the in-order PE never head-of-line blocks on an exp that hasn't
            # drained (each entry = one chunk's PVT matmul group)
            pending = collections.deque()

            def emit_attention_half(b, mh_i, kk, qq, v_big, ot, fillers=None,
                                    lag=1, rev=False, ndr=NDR, _ctr=[0]):
                mo0, mw0, mms, mcw = MH[mh_i]
                fillers = fillers if fillers is not None else []
                _ctr[0] += 1
                u = _ctr[0]
                ps_out = pvp.tile([128, 2, 512], F32, tag="pv", name=f"po{u}")
                zeroed = [False]

                def ensure_zero():
                    # zero-init each psum bank with a zero-stationary matmul:
                    # runs on PE (in-order with the PVT accumulation) and its
                    # full-bank write gives every start=False PVT matmul a
                    # tracked RAW dependency
                    if not zeroed[0]:
                        zeroed[0] = True
                        for q4 in range(2):
                            nc.tensor.matmul(ps_out[:, q4, :], zcol[:],
                                             zrow[:], start=True, stop=True,
                                             skip_group_check=True)

                # rev=True ends on a DoubleRow chunk (no DVE multiply in the
                # tail dependency chain) — used for the final half's drain
                order = list(reversed(range(NCH))) if rev else list(range(NCH))
                for idx, n in enumerate(order):
                    w = _cw(n)
                    st = stp.tile([128, 1024], F32, tag="st", name=f"st{u}_{n}")
                    for mo, mw in mms:
                        nc.tensor.matmul(
                            st[0:w, mo:mo + mw],
                            kk[:, 128 * n:128 * n + w],
                            qq[:, mo0 + mo:mo0 + mo + mw],
                            start=True, stop=(n >= ndr),
                        )
                    if n < ndr:
                        # rpe bias add: fp8 DoubleRow identity matmul, k-tile
                        # pair (n, n+1) with second tile zero-weighted (IZ),
                        # or (n-1, n) via ZI for the last DR chunk
                        lhs8 = iz_sb if n < ndr - 1 else zi_sb
                        n0 = n if n < ndr - 1 else n - 1
                        for mo, mw in mms:
                            nc.tensor.matmul(
                                st[0:w, mo:mo + mw],
                                lhs8[:, :, 0:w],
                                rpe8[:, n0:n0 + 2, mo0 + mo:mo0 + mo + mw],
                                start=False, stop=True, perf_mode=DR,
                                skip_group_check=True,
                            )
                    # drain gradually toward `lag` (at most 2 per chunk) so a
                    # lag transition doesn't batch PE work ahead of the next
                    # S matmul
                    target = max(lag, len(pending) - 2)
                    while len(pending) > target:
                        pending.popleft()()
                    pt = pt_pool.tile([128, 1024], F16, tag="pt", name=f"pt{u}_{n}")
                    nc.scalar.activation(
                        pt[0:w, 0:mw0], st[0:w, 0:mw0], Act.Exp,
                        bias=nbias[0:w])
                    if n >= ndr:
                        # rpe applied multiplicatively: pt *= exp(rpe) (DVE)
                        nc.vector.tensor_tensor(
                            out=pt[0:w, 0:mw0], in0=pt[0:w, 0:mw0],
                            in1=e16_sb[0:w, n - NDR, mo0:mo0 + mw0],
                            op=Alu.mult)
                    if debug and b == 0 and mh_i == 0 and n < 2:
                        nc.sync.dma_start(pt_dbg[n], pt[:])

                    def pvt(n=n, w=w, pt=pt, ps_out=ps_out,
                            last=(idx == NCH - 1)):
                        ensure_zero()
                        # transposed PV: stationary = pt m-chunk, moving = v_aug
                        for j, mp in enumerate(mcw):
                            q4, j4 = divmod(j, 4)
                            nc.tensor.matmul(
                                ps_out[0:mp, q4, 65 * j4:65 * j4 + 65],
                                pt[0:w, 128 * j:128 * j + mp],
                                v_big[0:w, n, :],
                                start=False, stop=last,
                                skip_group_check=True,
                            )
                        if last:
                            for q4 in range(2):
                                nc.vector.tensor_scalar(
                                    ot[:, 8 * mh_i + 4 * q4:
                                       8 * mh_i + 4 * q4 + 4, :],
                                    ps_out[:, q4, 0:260].rearrange(
                                        "p (c w) -> p c w", w=65),
                                    0.0, None, Alu.add)
                            nc.sync.dma_start(
                                out_d[b, :, 8 * mh_i:8 * mh_i + 8, :],
                                ot[:, 8 * mh_i:8 * mh_i + 8, :])
                    pending.append(pvt)
                    # interleave one next-batch conv/projection step per chunk
                    if fillers:
                        fillers.pop(0)()

            state, steps0 = emit_conv_proj(0, pe_conv=True)
            # prologue: run just enough projection for attention to start
            # (kv l-chunk 0 -> keys 0..499; q l-chunks 0..2 -> queries
            # 0..1500 covering m-half 0); the rest become fillers, with the
            # v^T transpose last and the first half's PVTs lagged behind it
            pre = steps0[0:2] + steps0[8:11]
            fill0 = steps0[2:8] + steps0[11:]
            for st_fn in pre:
                st_fn()
            load_late()
            for rep in range(repeat):
                for b in range(B):
                    kk, qq, v_big = state
                    ot = ob_pool.tile([128, NCH, 65], F32, tag="ot",
                                      name=f"ot{b}_{rep}")
                    if b + 1 < B or rep + 1 < repeat:
                        state, fillers = emit_conv_proj((b + 1) % B)
                    else:
                        fillers = []
                    first = (rep == 0 and b == 0)
                    if first:
                        load_late2()
                    # batch 0's first half is already resource-crunched:
                    # only its own remaining projection steps pop there;
                    # batch 1's steps wait for the second half
                    emit_attention_half(b, 0, kk, qq, v_big, ot,
                                        fill0 if first else fillers,
                                        lag=10 if first else 8,
                                        ndr=12 if first else NDR)
                    final = b + 1 >= B and rep + 1 >= repeat
                    emit_attention_half(b, 1, kk, qq, v_big, ot, fillers,
                                        lag=1, rev=final,
                                        ndr=12 if first else NDR)
                    for st_fn in fillers:
                        st_fn()
            while pending:
                pending.popleft()()
            if debug:
                nc.sync.dma_start(kk_dbg[:], state[0][:])
                nc.sync.dma_start(qq_dbg[:], state[1][:])
                nc.sync.dma_start(vb_dbg[:], state[2][:])

    nc.finalize()
    return nc


_NC_CACHE = None


def _get_nc():
    global _NC_CACHE
    if _NC_CACHE is None:
        _NC_CACHE = build_kernel()
    return _NC_CACHE


def _host_prep(x, rpe, Wq, bq, Wkv, bkv, Wl, bl):
    scale = float(HD) ** -0.5
    xt = np.ascontiguousarray(np.swapaxes(x, 1, 2))          # [B, C, L]
    xpad = np.zeros((B, C, PW), np.float16)
    xpad[:, :, 1:1 + CH] = xt[:, :, 0:CH]
    xpad[:, :, CH + 3:CH + 3 + CH] = xt[:, :, CH:L]

    w1 = Wl[:, 0, 0].astype(np.float64)
    w2 = Wl[:, 0, 1].astype(np.float64) + 1.0
    w3 = Wl[:, 0, 2].astype(np.float64)
    convw = np.zeros((128, 12), np.float32)
    for c in range(4):
        sl = slice(128 * c, 128 * c + 128)
        convw[:, 3 * c + 0] = w1[sl]
        convw[:, 3 * c + 1] = w2[sl]
        convw[:, 3 * c + 2] = w3[sl]

    bias_kv_full = (Wkv.astype(np.float64) @ bl.astype(np.float64)
                    + bkv.astype(np.float64))

    iz = np.zeros((128, 2, 128), E4)
    zi = np.zeros((128, 2, 128), E4)
    iz[:, 0][np.arange(128), np.arange(128)] = 1.0
    zi[:, 1][np.arange(128), np.arange(128)] = 1.0

    in_maps = []
    for h in range(H):
        r = slice(HD * h, HD * h + HD)
        rv = slice(C + HD * h, C + HD * h + HD)
        wqT = np.ascontiguousarray((Wq[r, :] * scale).T).astype(np.float16)
        wsel = np.concatenate([Wkv[r, :], Wkv[rv, :]], 0).astype(np.float64)
        wkvT = np.ascontiguousarray(wsel.T).astype(np.float16)
        taps = [w1, w2, w3]
        wkv3T = np.stack(
            [np.ascontiguousarray((wsel * taps[t][None, :]).T) for t in range(3)],
            0).astype(np.float16)
        biasq = (bq[r] * scale).astype(np.float32).reshape(HD, 1)
        biaskv = np.concatenate(
            [bias_kv_full[r], bias_kv_full[rv]]).astype(np.float32).reshape(128, 1)
        rpeT = np.zeros((NCH * 128, L), np.float32)
        rpeT[0:L] = rpe[0, h].T
        rpeTc = rpeT.reshape(NCH, 128, L)
        rpe8 = rpeTc.astype(E4)
        e16 = np.exp(rpeTc[NDR:NCH]).astype(np.float16)
        in_maps.append({
            "xpad": xpad, "rpe8": rpe8, "e16": e16, "iz": iz, "zi": zi,
            "wqT": wqT, "wkvT": wkvT, "wkv3T": wkv3T, "convw": convw,
            "biasq": biasq, "biaskv": biaskv,
        })
    return in_maps


def kernel(x, relative_pos_enc, Wq, bq, Wkv, bkv, Wl, bl):
    global LAST_EXEC_NS, LAST_RESULTS
    in_maps = _host_prep(np.asarray(x, np.float32),
                         np.asarray(relative_pos_enc, np.float32),
                         np.asarray(Wq, np.float32), np.asarray(bq, np.float32),
                         np.asarray(Wkv, np.float32), np.asarray(bkv, np.float32),
                         np.asarray(Wl, np.float32), np.asarray(bl, np.float32))
    nc = _get_nc()
    trace = bool(int(os.environ.get("KERNEL_TRACE", "0")))
    res = run_bass_kernel_spmd(nc, in_maps, core_ids=list(range(H)), trace=trace)
    LAST_EXEC_NS = res.exec_time_ns
    LAST_RESULTS = res
    arr = np.stack([res.results[h]["outT"] for h in range(H)], 0)
    # [H, B, 128, NCH, 65]: m = 128*chunk + p -> [H, B, L, 65]
    arr = arr.transpose(0, 1, 3, 2, 4).reshape(H, B, NCH * 128, 65)[:, :, 0:L]
    out_md = arr[:, :, :, 0:64] / arr[:, :, :, 64:65]
    out_t = np.ascontiguousarray(out_md.transpose(0, 1, 3, 2))  # [H, B, 64, L]
    out = np.ascontiguousarray(out_t.transpose(1, 0, 2, 3)).reshape(B, L, C)
    return out.astype(np.float32)
